# revision 14
# baseline (speedup 1.0000x reference)
"""ARMA GNN (3 layers, N=50000 nodes, E=800000 edges, F=256) on 8 TRN2 NeuronCores.

Strategy:
  - Shard nodes across 8 cores (6250 each); partition edges by destination owner
    so the segment-sum is local to each core.
  - All graph structure (edge lists, GCN norm) is known when the kernel is built,
    so the host precomputes: per-(src-half, dst-block) edge runs, int16 gather
    indices, and dense 128x128 "S matrices" (S[e, d] = norm_e one-hot on the dst
    column).  On device the whole message-passing step is:
        gather h[src] rows (SWDGE dma_gather)  ->  PSUM += S_chunk^T @ G_chunk
    i.e. gather + scale + segment-sum fused into TensorEngine matmuls.
  - The Q7 gather-descriptor generator costs ~3us per call regardless of size,
    so gather calls are packed to exactly 1024 indices, spanning dst-block
    boundaries (a straddling chunk simply feeds two matmuls with complementary
    zero columns).  Per-dst-block PSUM accumulators are spilled to SBUF between
    the two src-half passes and reloaded with an identity matmul.
  - Per layer: h = x @ Wi in bf16, AllGather'd in two chunks (src-half A then B,
    so half-A gathers overlap half-B's collective); message matmuls + x @ Wr
    accumulate in PSUM; transposed epilogue fuses ReLU+bias on the Act engine.
    x lives feature-major (xT) in SBUF between layers; host transposes output.
"""

import numpy as np
import ml_dtypes

import concourse.bass as bass
import concourse.bacc as bacc
import concourse.mybir as mybir
import concourse.tile as tile
from concourse.bass_utils import run_bass_kernel_spmd
from concourse.masks import make_identity

BF16 = ml_dtypes.bfloat16

# Problem constants (hardcoded per harness contract).
N = 50000
E = 800000
F = 256
L = 3
C = 8                     # cores
NL = N // C               # nodes per core = 6250
NB = (NL + 127) // 128    # dst blocks per core = 49
SA = 4096                 # local rows in src-half A (32 blocks)
SB_ = NL - SA             # local rows in src-half B = 2154 (17 blocks, last 106)
TBL = (C * SA, C * SB_)   # gather tables (32768, 17232) — int16-safe
NPAD = NB * 128           # padded local node count = 6272
GRING = 64                # G ring slots (chunks)


def _blkw(i):
    return NL - i * 128 if i == NB - 1 else 128


def _preprocess(x, edge_index, edge_attr, W_init, W_root, bias):
    """Host-side graph preprocessing. Returns (meta, per-core input maps)."""
    x = np.asarray(x, np.float32)
    ei = np.asarray(edge_index, np.int64)
    w = np.asarray(edge_attr, np.float32)
    W_init = np.asarray(W_init, np.float32)
    W_root = np.asarray(W_root, np.float32)
    bias = np.asarray(bias, np.float32)
    src, dst = ei[0], ei[1]

    deg = np.bincount(dst, weights=w.astype(np.float64), minlength=N).astype(np.float32)
    with np.errstate(divide="ignore"):
        dinv = np.where(deg > 0, 1.0 / np.sqrt(deg), 0.0).astype(np.float32)
    norm = (dinv[src] * w * dinv[dst]).astype(np.float32)

    core = dst // NL
    dloc = dst % NL
    db = dloc // 128
    dcol = dloc % 128
    sowner = src // NL
    sloc = src % NL
    half = (sloc >= SA).astype(np.int64)
    tbl = np.where(half == 0, sowner * SA + sloc, sowner * SB_ + (sloc - SA))

    # sort edges by (core, half, db, tbl)
    order = np.lexsort((tbl, db, half, core))
    g_core, g_half, g_db = core[order], half[order], db[order]
    g_tbl, g_norm, g_dcol = tbl[order], norm[order], dcol[order]

    # per-(core, half, db) counts -> unified run lengths (max over cores, SPMD)
    cnt = np.zeros((C, 2, NB), np.int64)
    np.add.at(cnt, (g_core, g_half, g_db), 1)
    Lhb = cnt.max(axis=0)                      # [2, NB]

    run_key = (g_core * 2 + g_half) * NB + g_db
    starts = np.searchsorted(run_key, np.arange(C * 2 * NB))
    ends = np.append(starts[1:], len(run_key))

    # unified layout: per half, concatenated padded (h, db) runs; each half's
    # total padded up to a chunk (128) multiple
    off_hb = np.zeros((2, NB), np.int64)
    POS = [0, 0]
    for h in range(2):
        p = 0
        for b in range(NB):
            off_hb[h, b] = p
            p += int(Lhb[h, b])
        POS[h] = -(-p // 128) * 128
    NCHUNK = [POS[0] // 128, POS[1] // 128]

    # pieces: (chunk, db, s0, s1) — intersection of a 128-chunk with a run.
    # Emitted in position order (runs are disjoint ascending intervals).
    pieces = [[], []]
    for h in range(2):
        for b in range(NB):
            lo, hi = int(off_hb[h, b]), int(off_hb[h, b] + Lhb[h, b])
            if lo == hi:
                continue
            for cc in range(lo // 128, (hi - 1) // 128 + 1):
                s0 = max(lo, cc * 128) - cc * 128
                s1 = min(hi, (cc + 1) * 128) - cc * 128
                pieces[h].append((cc, b, s0, s1))
        # position-ordered check (required for contiguous S streaming)
        ppos = [cc * 128 + s0 for (cc, b, s0, s1) in pieces[h]]
        assert all(ppos[i] < ppos[i + 1] for i in range(len(ppos) - 1))
    NPIECE = [len(pieces[0]), len(pieces[1])]
    SW = (NPIECE[0] + NPIECE[1]) * 128
    WT = (POS[0] + POS[1]) // 16

    # gather calls per half: groups of <=8 chunks
    calls = [[], []]
    for h in range(2):
        cc = 0
        while cc < NCHUNK[h]:
            n = min(8, NCHUNK[h] - cc)
            calls[h].append((cc, n))
            cc += n
    # max pieces per call (St tile sizing)
    maxpc = 1
    for h in range(2):
        cp = {}
        for pi, (cc, b, s0, s1) in enumerate(pieces[h]):
            cp.setdefault(cc // 8, []).append(pi)
        maxpc = max([maxpc] + [len(v) for v in cp.values()])

    in_maps = []
    for ci in range(C):
        idx_all = np.zeros((128, WT), np.int16)
        s_all = np.zeros((128, SW), BF16)
        for h in range(2):
            base = POS[0] // 16 if h == 1 else 0
            for b in range(NB):
                k = (ci * 2 + h) * NB + b
                a, e = starts[k], ends[k]
                ne = e - a
                if ne == 0:
                    continue
                j = int(off_hb[h, b]) + np.arange(ne)
                t16 = g_tbl[a:e].astype(np.int16)
                cols = base + j // 16
                rows = j % 16
                for grp in range(8):
                    idx_all[grp * 16 + rows, cols] = t16
        poff = 0
        for h in range(2):
            for (cc, b, s0, s1) in pieces[h]:
                k = (ci * 2 + h) * NB + b
                a, e = starts[k], ends[k]
                ne = e - a
                glo = cc * 128 + s0 - int(off_hb[h, b])
                ghi = cc * 128 + s1 - int(off_hb[h, b])
                lo, hi = max(0, glo), min(ne, ghi)
                if hi > lo:
                    jj = np.arange(lo, hi)
                    prow = (int(off_hb[h, b]) + jj) % 128
                    s_all[prow, poff + g_dcol[a + lo:a + hi]] = \
                        g_norm[a + lo:a + hi].astype(BF16)
                poff += 128
        xT = np.zeros((256, NPAD), BF16)
        xT[:, :NL] = x[ci * NL:(ci + 1) * NL].T.astype(BF16)
        in_maps.append(dict(
            xT=xT,
            idx_all=idx_all,
            s_all=s_all,
            wi=W_init.reshape(L, 2, 128, F).astype(BF16),
            wr=W_root.reshape(L, 2, 128, F).astype(BF16),
            bias_c=np.ascontiguousarray(
                bias.reshape(L * 2, 128).T.astype(np.float32)),  # [128, L*2]
        ))

    meta = dict(pieces=pieces, calls=calls, NCHUNK=NCHUNK, NPIECE=NPIECE,
                POS=POS, WT=WT, SW=SW, maxpc=maxpc)
    return meta, in_maps


def _h_block(nc, psp, wp, l, nb, xsrc, wi_sb, hb):
    """Emit h = x @ Wi for one node block of layer l into its bounce buffer."""
    bf = mybir.dt.bfloat16
    f32 = mybir.dt.float32
    hbA, hbB = hb
    w = _blkw(nb)
    c0 = nb * 128
    ph = psp.tile([128, F], f32, tag="ph", bufs=2, name=f"ph{l}_{nb}")
    for g in range(2):
        nc.tensor.matmul(out=ph[:w, :], lhsT=xsrc[g][:, c0:c0 + w],
                         rhs=wi_sb[l][g][:], start=(g == 0), stop=(g == 1))
    hsb = wp.tile([128, F], bf, tag="hsb", name=f"hsb{l}_{nb}")
    nc.vector.tensor_copy(hsb[:w, :], ph[:w, :])
    if nb < SA // 128:
        nc.sync.dma_start(out=hbA[c0:c0 + w, :], in_=hsb[:w, :])
    else:
        r0 = c0 - SA
        nc.sync.dma_start(out=hbB[r0:r0 + w, :], in_=hsb[:w, :])


def _epilogue(nc, psp, wp, t, b, l, ident, bias_sb, xw_, outT, nxt):
    bf = mybir.dt.bfloat16
    f32 = mybir.dt.float32
    wd = _blkw(b)
    c0 = b * 128
    cp = wp.tile([128, F], bf, tag="cp", bufs=3, name=f"cp{l}_{b}")
    nc.vector.tensor_copy(cp[:], t[:])
    for g in range(2):
        pt = psp.tile([128, 128], bf, tag="pt", bufs=2, name=f"pt{l}_{b}_{g}")
        nc.tensor.transpose(out=pt[:], in_=cp[:, g * 128:(g + 1) * 128],
                            identity=ident[:])
        bcol = bias_sb[l * 2 + g][:]
        if l < L - 1:
            nc.scalar.activation(
                out=xw_[g][:, c0:c0 + wd], in_=pt[:, :wd],
                func=mybir.ActivationFunctionType.Relu, bias=bcol)
        else:
            ot = wp.tile([128, 128], f32, tag="ot", bufs=2, name=f"ot{l}_{b}_{g}")
            nc.scalar.activation(
                out=ot[:, :wd], in_=pt[:, :wd],
                func=mybir.ActivationFunctionType.Relu, bias=bcol)
            nc.sync.dma_start(out=outT[g][:, c0:c0 + wd], in_=ot[:, :wd])
    if l < L - 1:
        # queue next layer's h for this block; emitted with a lag so the PE
        # doesn't stall waiting on this epilogue's Act write
        nxt["pend"].append(b)
        _flush_h(nc, psp, wp, l, xw_, nxt, lag=4)


def _flush_h(nc, psp, wp, l, xw_, nxt, lag):
    while len(nxt["pend"]) > lag:
        b = nxt["pend"].pop(0)
        _h_block(nc, psp, wp, l + 1, b, xw_, nxt["wi_sb"], nxt["hb"])
        if b < SA // 128:
            nxt["remA"] -= 1
            if nxt["remA"] == 0:
                nc.gpsimd.collective_compute(
                    "AllGather", mybir.AluOpType.bypass,
                    replica_groups=nxt["groups"], ins=[nxt["hb"][0][:]],
                    outs=[nxt["hg"][0][:]])
        else:
            nxt["remB"] -= 1
            if nxt["remB"] == 0:
                nc.gpsimd.collective_compute(
                    "AllGather", mybir.AluOpType.bypass,
                    replica_groups=nxt["groups"], ins=[nxt["hb"][1][:]],
                    outs=[nxt["hg"][1][:]])


def _build(meta):
    pieces, calls = meta["pieces"], meta["calls"]
    POS, WT, SW = meta["POS"], meta["WT"], meta["SW"]
    NPIECE, maxpc = meta["NPIECE"], meta["maxpc"]
    bf = mybir.dt.bfloat16
    f32 = mybir.dt.float32

    nc = bacc.Bacc("TRN2", target_bir_lowering=False, debug=False, num_devices=C,
                   num_swdge_queues=4)
    xT_p = nc.dram_tensor("xT", [256, NPAD], bf, kind="ExternalInput")
    idx_p = nc.dram_tensor("idx_all", [128, WT], mybir.dt.int16, kind="ExternalInput")
    s_p = nc.dram_tensor("s_all", [128, SW], bf, kind="ExternalInput")
    wi_p = nc.dram_tensor("wi", [L, 2, 128, F], bf, kind="ExternalInput")
    wr_p = nc.dram_tensor("wr", [L, 2, 128, F], bf, kind="ExternalInput")
    bias_p = nc.dram_tensor("bias_c", [128, L * 2], f32, kind="ExternalInput")
    outT = [nc.dram_tensor(f"outT{g}", [128, NL], f32, kind="ExternalOutput")
            for g in range(2)]

    groups = [list(range(C))]
    # chunk -> [(local_pi, cc, b, s0, s1)] and db -> last local_pi, per half
    chunk_pieces = [{}, {}]
    db_last = [[None] * NB, [None] * NB]
    db_any = [[False] * NB, [False] * NB]
    for h in range(2):
        for pi, (cc, b, s0, s1) in enumerate(pieces[h]):
            chunk_pieces[h].setdefault(cc, []).append((pi, cc, b))
            db_last[h][b] = pi
            db_any[h][b] = True

    with tile.TileContext(nc) as tc:
        with (
            tc.tile_pool(name="persist", bufs=1) as pp,
            tc.tile_pool(name="dram", bufs=2, space="DRAM") as dp,
            tc.tile_pool(name="psum", bufs=3, space="PSUM") as psp,
            tc.tile_pool(name="work", bufs=3) as wp,
        ):
            ident = pp.tile([128, 128], bf)
            make_identity(nc, ident[:])
            idx_sb = pp.tile([128, WT], mybir.dt.int16)
            nc.sync.dma_start(out=idx_sb[:], in_=idx_p[:])
            bias_sb = [pp.tile([128, 1], f32, name=f"bias{c}") for c in range(L * 2)]
            for c_ in range(L * 2):
                nc.sync.dma_start(out=bias_sb[c_][:], in_=bias_p[:, c_:c_ + 1])
            wi_sb = [[pp.tile([128, F], bf, name=f"wi{l}{g}") for g in range(2)]
                     for l in range(L)]
            wr_sb = [[pp.tile([128, F], bf, name=f"wr{l}{g}") for g in range(2)]
                     for l in range(L)]
            for l in range(L):
                for g in range(2):
                    nc.sync.dma_start(out=wi_sb[l][g][:], in_=wi_p[l, g])
                    nc.sync.dma_start(out=wr_sb[l][g][:], in_=wr_p[l, g])
            xa = [pp.tile([128, NPAD], bf, name=f"xa{g}") for g in range(2)]
            xb = [pp.tile([128, NPAD], bf, name=f"xb{g}") for g in range(2)]
            for g in range(2):
                nc.sync.dma_start(out=xa[g][:], in_=xT_p[g * 128:(g + 1) * 128, :])
                if NPAD > NL:
                    nc.gpsimd.memset(xb[g][:, NL:], 0.0)
            acc_sb = pp.tile([128, NB * F], bf)   # pass A -> pass B spill

            # per-layer bounce/gather-table tiles (bufs=2 ping-pong)
            hb = []
            hg = []
            for l in range(L):
                hb.append((dp.tile([SA, F], bf, tag="hbA", name=f"hbA{l}"),
                           dp.tile([SB_, F], bf, tag="hbB", name=f"hbB{l}")))
                hg.append((dp.tile([TBL[0], F], bf, addr_space="Shared",
                                   tag="hgA", name=f"hgA{l}"),
                           dp.tile([TBL[1], F], bf, addr_space="Shared",
                                   tag="hgB", name=f"hgB{l}")))

            # prologue: layer 0 h-phase + collectives
            for nb in range(NB):
                _h_block(nc, psp, wp, 0, nb, xa, wi_sb, hb[0])
                if nb == SA // 128 - 1:
                    nc.gpsimd.collective_compute(
                        "AllGather", mybir.AluOpType.bypass,
                        replica_groups=groups, ins=[hb[0][0][:]],
                        outs=[hg[0][0][:]])
            nc.gpsimd.collective_compute(
                "AllGather", mybir.AluOpType.bypass,
                replica_groups=groups, ins=[hb[0][1][:]], outs=[hg[0][1][:]])

            qn = [0]
            for l in range(L):
                xr_ = xa if l % 2 == 0 else xb
                xw_ = xb if l % 2 == 0 else xa
                hgA, hgB = hg[l]
                if l < L - 1:
                    nxt = dict(wi_sb=wi_sb, hb=hb[l + 1], hg=hg[l + 1],
                               remA=SA // 128, remB=NB - SA // 128,
                               groups=groups, pend=[])
                else:
                    nxt = None

                # ---- message passing: pass A then pass B ----
                Gr = pp.tile([128, GRING, F], bf, name=f"Gr{l}", tag="Gr")
                pa = {}
                for h in range(2):
                    hgx = hgA if h == 0 else hgB
                    ibase = POS[0] // 16 if h == 1 else 0
                    pbase = NPIECE[0] if h == 1 else 0
                    for (clo, ncnk) in calls[h]:
                        slot0 = clo % GRING
                        nidx = ncnk * 128
                        nc.gpsimd.dma_gather(
                            out_ap=Gr[:, slot0:slot0 + ncnk, :], in_ap=hgx[:],
                            idxs_ap=idx_sb[:, ibase + clo * 8:
                                           ibase + (clo + ncnk) * 8],
                            num_idxs=nidx, num_idxs_reg=nidx,
                            elem_size=F, queue_num=qn[0] % 4)
                        qn[0] += 1
                        plist = []
                        for cc in range(clo, clo + ncnk):
                            plist += chunk_pieces[h].get(cc, [])
                        if not plist:
                            continue
                        p0 = plist[0][0]
                        np_ = len(plist)
                        St = wp.tile([128, maxpc * 128], bf, tag="St", bufs=4,
                                     name=f"St{l}_{h}_{clo}")
                        nc.sync.dma_start(
                            out=St[:, :np_ * 128],
                            in_=s_p[:, (pbase + p0) * 128:(pbase + p0 + np_) * 128])
                        for k, (pi, cc, b) in enumerate(plist):
                            if b not in pa:
                                t = psp.tile([128, F], f32, tag="pa", bufs=3,
                                             name=f"pa{l}_{h}_{b}")
                                pa[b] = t
                                if h == 1 and db_any[0][b]:
                                    nc.tensor.matmul(
                                        out=t[:], lhsT=ident[:],
                                        rhs=acc_sb[:, b * F:(b + 1) * F],
                                        start=True, stop=False)
                                else:
                                    for g in range(2):
                                        nc.tensor.matmul(
                                            out=t[:],
                                            lhsT=xr_[g][:, b * 128:b * 128 + 128],
                                            rhs=wr_sb[l][g][:],
                                            start=(g == 0), stop=False)
                            last_piece = (pi == db_last[h][b])
                            nc.tensor.matmul(
                                out=pa[b][:],
                                lhsT=St[:, k * 128:(k + 1) * 128],
                                rhs=Gr[:, slot0 + (cc - clo), :],
                                start=False, stop=last_piece)
                            if last_piece:
                                t = pa.pop(b)
                                if h == 0 and db_any[1][b]:
                                    nc.vector.tensor_copy(
                                        acc_sb[:, b * F:(b + 1) * F], t[:])
                                else:
                                    _epilogue(nc, psp, wp, t, b, l, ident,
                                              bias_sb, xw_, outT, nxt)
                assert not pa
                if nxt is not None:
                    _flush_h(nc, psp, wp, l, xw_, nxt, lag=0)
                # dbs with no edges at all (xr + bias + relu only)
                for b in range(NB):
                    if not db_any[0][b] and not db_any[1][b]:
                        t = psp.tile([128, F], f32, tag="pa", bufs=3,
                                     name=f"paz{l}_{b}")
                        for g in range(2):
                            nc.tensor.matmul(
                                out=t[:], lhsT=xr_[g][:, b * 128:b * 128 + 128],
                                rhs=wr_sb[l][g][:], start=(g == 0), stop=(g == 1))
                        _epilogue(nc, psp, wp, t, b, l, ident, bias_sb,
                                  xw_, outT, nxt)
    nc.compile()
    return nc


_CACHE = {}


def kernel(**inputs):
    meta, in_maps = _preprocess(**inputs)
    key = (tuple(map(tuple, meta["calls"][0])), tuple(map(tuple, meta["calls"][1])),
           tuple(map(tuple, meta["pieces"][0])), tuple(map(tuple, meta["pieces"][1])))
    nc = _CACHE.get(key)
    if nc is None:
        nc = _build(meta)
        _CACHE[key] = nc
    res = run_bass_kernel_spmd(nc, in_maps, list(range(C)), trace=False)
    out = np.empty((N, F), np.float32)
    for ci in range(C):
        r = res.results[ci]
        xt = np.concatenate([r["outT0"], r["outT1"]], axis=0)  # [256, NL]
        out[ci * NL:(ci + 1) * NL] = xt.T
    return out


# revision 15
# speedup vs baseline: 1.0081x; 1.0081x over previous
"""ARMA GNN (3 layers, N=50000 nodes, E=800000 edges, F=256) on 8 TRN2 NeuronCores.

Strategy:
  - Shard nodes across 8 cores (6250 each); partition edges by destination owner
    so the segment-sum is local to each core.
  - All graph structure (edge lists, GCN norm) is known when the kernel is built,
    so the host precomputes: per-(src-half, dst-block) edge runs, int16 gather
    indices, and dense 128x128 "S matrices" (S[e, d] = norm_e one-hot on the dst
    column).  On device the whole message-passing step is:
        gather h[src] rows (SWDGE dma_gather)  ->  PSUM += S_chunk^T @ G_chunk
    i.e. gather + scale + segment-sum fused into TensorEngine matmuls.
  - The Q7 gather-descriptor generator costs ~3us per call regardless of size,
    so gather calls are packed to exactly 1024 indices, spanning dst-block
    boundaries (a straddling chunk simply feeds two matmuls with complementary
    zero columns).  Per-dst-block PSUM accumulators are spilled to SBUF between
    the two src-half passes and reloaded with an identity matmul.
  - Per layer: h = x @ Wi in bf16, AllGather'd in two chunks (src-half A then B,
    so half-A gathers overlap half-B's collective); message matmuls + x @ Wr
    accumulate in PSUM; transposed epilogue fuses ReLU+bias on the Act engine.
    x lives feature-major (xT) in SBUF between layers; host transposes output.
"""

import numpy as np
import ml_dtypes

import concourse.bass as bass
import concourse.bacc as bacc
import concourse.mybir as mybir
import concourse.tile as tile
from concourse.bass_utils import run_bass_kernel_spmd
from concourse.masks import make_identity

BF16 = ml_dtypes.bfloat16

# Problem constants (hardcoded per harness contract).
N = 50000
E = 800000
F = 256
L = 3
C = 8                     # cores
NL = N // C               # nodes per core = 6250
NB = (NL + 127) // 128    # dst blocks per core = 49
SA = 4096                 # local rows in src-half A (32 blocks)
SB_ = NL - SA             # local rows in src-half B = 2154 (17 blocks, last 106)
TBL = (C * SA, C * SB_)   # gather tables (32768, 17232) — int16-safe
NPAD = NB * 128           # padded local node count = 6272
GRING = 64                # G ring slots (chunks)


def _blkw(i):
    return NL - i * 128 if i == NB - 1 else 128


def _preprocess(x, edge_index, edge_attr, W_init, W_root, bias):
    """Host-side graph preprocessing. Returns (meta, per-core input maps)."""
    x = np.asarray(x, np.float32)
    ei = np.asarray(edge_index, np.int64)
    w = np.asarray(edge_attr, np.float32)
    W_init = np.asarray(W_init, np.float32)
    W_root = np.asarray(W_root, np.float32)
    bias = np.asarray(bias, np.float32)
    src, dst = ei[0], ei[1]

    deg = np.bincount(dst, weights=w.astype(np.float64), minlength=N).astype(np.float32)
    with np.errstate(divide="ignore"):
        dinv = np.where(deg > 0, 1.0 / np.sqrt(deg), 0.0).astype(np.float32)
    norm = (dinv[src] * w * dinv[dst]).astype(np.float32)

    core = dst // NL
    dloc = dst % NL
    db = dloc // 128
    dcol = dloc % 128
    sowner = src // NL
    sloc = src % NL
    half = (sloc >= SA).astype(np.int64)
    tbl = np.where(half == 0, sowner * SA + sloc, sowner * SB_ + (sloc - SA))

    # sort edges by (core, half, db, tbl)
    order = np.lexsort((tbl, db, half, core))
    g_core, g_half, g_db = core[order], half[order], db[order]
    g_tbl, g_norm, g_dcol = tbl[order], norm[order], dcol[order]

    # per-(core, half, db) counts -> unified run lengths (max over cores, SPMD)
    cnt = np.zeros((C, 2, NB), np.int64)
    np.add.at(cnt, (g_core, g_half, g_db), 1)
    Lhb = cnt.max(axis=0)                      # [2, NB]

    run_key = (g_core * 2 + g_half) * NB + g_db
    starts = np.searchsorted(run_key, np.arange(C * 2 * NB))
    ends = np.append(starts[1:], len(run_key))

    # unified layout: per half, concatenated padded (h, db) runs; each half's
    # total padded up to a chunk (128) multiple
    off_hb = np.zeros((2, NB), np.int64)
    POS = [0, 0]
    for h in range(2):
        p = 0
        for b in range(NB):
            off_hb[h, b] = p
            p += int(Lhb[h, b])
        POS[h] = -(-p // 128) * 128
    NCHUNK = [POS[0] // 128, POS[1] // 128]

    # pieces: (chunk, db, s0, s1) — intersection of a 128-chunk with a run.
    # Emitted in position order (runs are disjoint ascending intervals).
    pieces = [[], []]
    for h in range(2):
        for b in range(NB):
            lo, hi = int(off_hb[h, b]), int(off_hb[h, b] + Lhb[h, b])
            if lo == hi:
                continue
            for cc in range(lo // 128, (hi - 1) // 128 + 1):
                s0 = max(lo, cc * 128) - cc * 128
                s1 = min(hi, (cc + 1) * 128) - cc * 128
                pieces[h].append((cc, b, s0, s1))
        # position-ordered check (required for contiguous S streaming)
        ppos = [cc * 128 + s0 for (cc, b, s0, s1) in pieces[h]]
        assert all(ppos[i] < ppos[i + 1] for i in range(len(ppos) - 1))
    NPIECE = [len(pieces[0]), len(pieces[1])]
    SW = (NPIECE[0] + NPIECE[1]) * 128
    WT = (POS[0] + POS[1]) // 16

    # gather calls per half: groups of <=8 chunks
    calls = [[], []]
    for h in range(2):
        cc = 0
        while cc < NCHUNK[h]:
            n = min(8, NCHUNK[h] - cc)
            calls[h].append((cc, n))
            cc += n
    # max pieces per call (St tile sizing)
    maxpc = 1
    for h in range(2):
        cp = {}
        for pi, (cc, b, s0, s1) in enumerate(pieces[h]):
            cp.setdefault(cc // 8, []).append(pi)
        maxpc = max([maxpc] + [len(v) for v in cp.values()])

    in_maps = []
    for ci in range(C):
        idx_all = np.zeros((128, WT), np.int16)
        s_all = np.zeros((128, SW), BF16)
        for h in range(2):
            base = POS[0] // 16 if h == 1 else 0
            for b in range(NB):
                k = (ci * 2 + h) * NB + b
                a, e = starts[k], ends[k]
                ne = e - a
                if ne == 0:
                    continue
                j = int(off_hb[h, b]) + np.arange(ne)
                t16 = g_tbl[a:e].astype(np.int16)
                cols = base + j // 16
                rows = j % 16
                for grp in range(8):
                    idx_all[grp * 16 + rows, cols] = t16
        poff = 0
        for h in range(2):
            for (cc, b, s0, s1) in pieces[h]:
                k = (ci * 2 + h) * NB + b
                a, e = starts[k], ends[k]
                ne = e - a
                glo = cc * 128 + s0 - int(off_hb[h, b])
                ghi = cc * 128 + s1 - int(off_hb[h, b])
                lo, hi = max(0, glo), min(ne, ghi)
                if hi > lo:
                    jj = np.arange(lo, hi)
                    prow = (int(off_hb[h, b]) + jj) % 128
                    s_all[prow, poff + g_dcol[a + lo:a + hi]] = \
                        g_norm[a + lo:a + hi].astype(BF16)
                poff += 128
        xT = np.zeros((256, NPAD), BF16)
        xT[:, :NL] = x[ci * NL:(ci + 1) * NL].T.astype(BF16)
        in_maps.append(dict(
            xT=xT,
            idx_all=idx_all,
            s_all=s_all,
            wi=W_init.reshape(L, 2, 128, F).astype(BF16),
            wr=W_root.reshape(L, 2, 128, F).astype(BF16),
            bias_c=np.ascontiguousarray(
                bias.reshape(L * 2, 128).T.astype(np.float32)),  # [128, L*2]
        ))

    meta = dict(pieces=pieces, calls=calls, NCHUNK=NCHUNK, NPIECE=NPIECE,
                POS=POS, WT=WT, SW=SW, maxpc=maxpc)
    return meta, in_maps


def _h_block(nc, psp, wp, l, nb, xsrc, wi_sb, hb):
    """Emit h = x @ Wi for one node block of layer l into its bounce buffer."""
    bf = mybir.dt.bfloat16
    f32 = mybir.dt.float32
    hbA, hbB = hb
    w = _blkw(nb)
    c0 = nb * 128
    ph = psp.tile([128, F], f32, tag="ph", bufs=2, name=f"ph{l}_{nb}")
    for g in range(2):
        nc.tensor.matmul(out=ph[:w, :], lhsT=xsrc[g][:, c0:c0 + w],
                         rhs=wi_sb[l][g][:], start=(g == 0), stop=(g == 1))
    hsb = wp.tile([128, F], bf, tag="hsb", name=f"hsb{l}_{nb}")
    nc.vector.tensor_copy(hsb[:w, :], ph[:w, :])
    if nb < SA // 128:
        nc.sync.dma_start(out=hbA[c0:c0 + w, :], in_=hsb[:w, :])
    else:
        r0 = c0 - SA
        nc.sync.dma_start(out=hbB[r0:r0 + w, :], in_=hsb[:w, :])


def _epilogue(nc, psp, wp, t, b, l, ident, bias_sb, xw_, outT, nxt):
    bf = mybir.dt.bfloat16
    f32 = mybir.dt.float32
    wd = _blkw(b)
    c0 = b * 128
    cp = wp.tile([128, F], bf, tag="cp", bufs=3, name=f"cp{l}_{b}")
    nc.vector.tensor_copy(cp[:], t[:])
    for g in range(2):
        pt = psp.tile([128, 128], bf, tag="pt", bufs=2, name=f"pt{l}_{b}_{g}")
        nc.tensor.transpose(out=pt[:], in_=cp[:, g * 128:(g + 1) * 128],
                            identity=ident[:])
        bcol = bias_sb[l * 2 + g][:]
        if l < L - 1:
            nc.scalar.activation(
                out=xw_[g][:, c0:c0 + wd], in_=pt[:, :wd],
                func=mybir.ActivationFunctionType.Relu, bias=bcol)
        else:
            ot = wp.tile([128, 128], f32, tag="ot", bufs=2, name=f"ot{l}_{b}_{g}")
            nc.scalar.activation(
                out=ot[:, :wd], in_=pt[:, :wd],
                func=mybir.ActivationFunctionType.Relu, bias=bcol)
            nc.sync.dma_start(out=outT[g][:, c0:c0 + wd], in_=ot[:, :wd])
    if l < L - 1:
        # queue next layer's h for this block; emitted with a lag so the PE
        # doesn't stall waiting on this epilogue's Act write
        nxt["pend"].append(b)
        _flush_h(nc, psp, wp, l, xw_, nxt, lag=4)


def _flush_h(nc, psp, wp, l, xw_, nxt, lag):
    while len(nxt["pend"]) > lag:
        b = nxt["pend"].pop(0)
        _h_block(nc, psp, wp, l + 1, b, xw_, nxt["wi_sb"], nxt["hb"])
        if b < SA // 128:
            nxt["remA"] -= 1
            if nxt["remA"] == 0:
                nc.gpsimd.collective_compute(
                    "AllGather", mybir.AluOpType.bypass,
                    replica_groups=nxt["groups"], ins=[nxt["hb"][0][:]],
                    outs=[nxt["hg"][0][:]])
        else:
            nxt["remB"] -= 1
            if nxt["remB"] == 0:
                nc.gpsimd.collective_compute(
                    "AllGather", mybir.AluOpType.bypass,
                    replica_groups=nxt["groups"], ins=[nxt["hb"][1][:]],
                    outs=[nxt["hg"][1][:]])


def _build(meta):
    pieces, calls = meta["pieces"], meta["calls"]
    POS, WT, SW = meta["POS"], meta["WT"], meta["SW"]
    NPIECE, maxpc = meta["NPIECE"], meta["maxpc"]
    bf = mybir.dt.bfloat16
    f32 = mybir.dt.float32

    nc = bacc.Bacc("TRN2", target_bir_lowering=False, debug=False, num_devices=C,
                   num_swdge_queues=4, dynamic_dma_scratch_size=32768)
    xT_p = nc.dram_tensor("xT", [256, NPAD], bf, kind="ExternalInput")
    idx_p = nc.dram_tensor("idx_all", [128, WT], mybir.dt.int16, kind="ExternalInput")
    s_p = nc.dram_tensor("s_all", [128, SW], bf, kind="ExternalInput")
    wi_p = nc.dram_tensor("wi", [L, 2, 128, F], bf, kind="ExternalInput")
    wr_p = nc.dram_tensor("wr", [L, 2, 128, F], bf, kind="ExternalInput")
    bias_p = nc.dram_tensor("bias_c", [128, L * 2], f32, kind="ExternalInput")
    outT = [nc.dram_tensor(f"outT{g}", [128, NL], f32, kind="ExternalOutput")
            for g in range(2)]

    groups = [list(range(C))]
    # chunk -> [(local_pi, cc, b, s0, s1)] and db -> last local_pi, per half
    chunk_pieces = [{}, {}]
    db_last = [[None] * NB, [None] * NB]
    db_any = [[False] * NB, [False] * NB]
    for h in range(2):
        for pi, (cc, b, s0, s1) in enumerate(pieces[h]):
            chunk_pieces[h].setdefault(cc, []).append((pi, cc, b))
            db_last[h][b] = pi
            db_any[h][b] = True

    with tile.TileContext(nc) as tc:
        with (
            tc.tile_pool(name="persist", bufs=1) as pp,
            tc.tile_pool(name="dram", bufs=2, space="DRAM") as dp,
            tc.tile_pool(name="psum", bufs=3, space="PSUM") as psp,
            tc.tile_pool(name="work", bufs=3) as wp,
        ):
            ident = pp.tile([128, 128], bf)
            make_identity(nc, ident[:])
            idx_sb = pp.tile([128, WT], mybir.dt.int16)
            nc.sync.dma_start(out=idx_sb[:], in_=idx_p[:])
            bias_sb = [pp.tile([128, 1], f32, name=f"bias{c}") for c in range(L * 2)]
            for c_ in range(L * 2):
                nc.sync.dma_start(out=bias_sb[c_][:], in_=bias_p[:, c_:c_ + 1])
            wi_sb = [[pp.tile([128, F], bf, name=f"wi{l}{g}") for g in range(2)]
                     for l in range(L)]
            wr_sb = [[pp.tile([128, F], bf, name=f"wr{l}{g}") for g in range(2)]
                     for l in range(L)]
            for l in range(L):
                for g in range(2):
                    nc.sync.dma_start(out=wi_sb[l][g][:], in_=wi_p[l, g])
                    nc.sync.dma_start(out=wr_sb[l][g][:], in_=wr_p[l, g])
            xa = [pp.tile([128, NPAD], bf, name=f"xa{g}") for g in range(2)]
            xb = [pp.tile([128, NPAD], bf, name=f"xb{g}") for g in range(2)]
            for g in range(2):
                nc.sync.dma_start(out=xa[g][:], in_=xT_p[g * 128:(g + 1) * 128, :])
                if NPAD > NL:
                    nc.gpsimd.memset(xb[g][:, NL:], 0.0)
            acc_sb = pp.tile([128, NB * F], bf)   # pass A -> pass B spill

            # per-layer bounce/gather-table tiles (bufs=2 ping-pong)
            hb = []
            hg = []
            for l in range(L):
                hb.append((dp.tile([SA, F], bf, tag="hbA", name=f"hbA{l}"),
                           dp.tile([SB_, F], bf, tag="hbB", name=f"hbB{l}")))
                hg.append((dp.tile([TBL[0], F], bf, addr_space="Shared",
                                   tag="hgA", name=f"hgA{l}"),
                           dp.tile([TBL[1], F], bf, addr_space="Shared",
                                   tag="hgB", name=f"hgB{l}")))

            # prologue: layer 0 h-phase + collectives
            for nb in range(NB):
                _h_block(nc, psp, wp, 0, nb, xa, wi_sb, hb[0])
                if nb == SA // 128 - 1:
                    nc.gpsimd.collective_compute(
                        "AllGather", mybir.AluOpType.bypass,
                        replica_groups=groups, ins=[hb[0][0][:]],
                        outs=[hg[0][0][:]])
            nc.gpsimd.collective_compute(
                "AllGather", mybir.AluOpType.bypass,
                replica_groups=groups, ins=[hb[0][1][:]], outs=[hg[0][1][:]])

            qn = [0]
            for l in range(L):
                xr_ = xa if l % 2 == 0 else xb
                xw_ = xb if l % 2 == 0 else xa
                hgA, hgB = hg[l]
                if l < L - 1:
                    nxt = dict(wi_sb=wi_sb, hb=hb[l + 1], hg=hg[l + 1],
                               remA=SA // 128, remB=NB - SA // 128,
                               groups=groups, pend=[])
                else:
                    nxt = None

                # ---- message passing: pass A then pass B ----
                Gr = pp.tile([128, GRING, F], bf, name=f"Gr{l}", tag="Gr")
                pa = {}
                for h in range(2):
                    hgx = hgA if h == 0 else hgB
                    ibase = POS[0] // 16 if h == 1 else 0
                    pbase = NPIECE[0] if h == 1 else 0
                    for (clo, ncnk) in calls[h]:
                        slot0 = clo % GRING
                        nidx = ncnk * 128
                        nc.gpsimd.dma_gather(
                            out_ap=Gr[:, slot0:slot0 + ncnk, :], in_ap=hgx[:],
                            idxs_ap=idx_sb[:, ibase + clo * 8:
                                           ibase + (clo + ncnk) * 8],
                            num_idxs=nidx, num_idxs_reg=nidx,
                            elem_size=F, queue_num=qn[0] % 4)
                        qn[0] += 1
                        plist = []
                        for cc in range(clo, clo + ncnk):
                            plist += chunk_pieces[h].get(cc, [])
                        if not plist:
                            continue
                        p0 = plist[0][0]
                        np_ = len(plist)
                        St = wp.tile([128, maxpc * 128], bf, tag="St", bufs=4,
                                     name=f"St{l}_{h}_{clo}")
                        nc.sync.dma_start(
                            out=St[:, :np_ * 128],
                            in_=s_p[:, (pbase + p0) * 128:(pbase + p0 + np_) * 128])
                        for k, (pi, cc, b) in enumerate(plist):
                            if b not in pa:
                                t = psp.tile([128, F], f32, tag="pa", bufs=3,
                                             name=f"pa{l}_{h}_{b}")
                                pa[b] = t
                                if h == 1 and db_any[0][b]:
                                    nc.tensor.matmul(
                                        out=t[:], lhsT=ident[:],
                                        rhs=acc_sb[:, b * F:(b + 1) * F],
                                        start=True, stop=False)
                                else:
                                    for g in range(2):
                                        nc.tensor.matmul(
                                            out=t[:],
                                            lhsT=xr_[g][:, b * 128:b * 128 + 128],
                                            rhs=wr_sb[l][g][:],
                                            start=(g == 0), stop=False)
                            last_piece = (pi == db_last[h][b])
                            nc.tensor.matmul(
                                out=pa[b][:],
                                lhsT=St[:, k * 128:(k + 1) * 128],
                                rhs=Gr[:, slot0 + (cc - clo), :],
                                start=False, stop=last_piece)
                            if last_piece:
                                t = pa.pop(b)
                                if h == 0 and db_any[1][b]:
                                    nc.vector.tensor_copy(
                                        acc_sb[:, b * F:(b + 1) * F], t[:])
                                else:
                                    _epilogue(nc, psp, wp, t, b, l, ident,
                                              bias_sb, xw_, outT, nxt)
                assert not pa
                if nxt is not None:
                    _flush_h(nc, psp, wp, l, xw_, nxt, lag=0)
                # dbs with no edges at all (xr + bias + relu only)
                for b in range(NB):
                    if not db_any[0][b] and not db_any[1][b]:
                        t = psp.tile([128, F], f32, tag="pa", bufs=3,
                                     name=f"paz{l}_{b}")
                        for g in range(2):
                            nc.tensor.matmul(
                                out=t[:], lhsT=xr_[g][:, b * 128:b * 128 + 128],
                                rhs=wr_sb[l][g][:], start=(g == 0), stop=(g == 1))
                        _epilogue(nc, psp, wp, t, b, l, ident, bias_sb,
                                  xw_, outT, nxt)
    nc.compile()
    return nc


_CACHE = {}


def kernel(**inputs):
    meta, in_maps = _preprocess(**inputs)
    key = (tuple(map(tuple, meta["calls"][0])), tuple(map(tuple, meta["calls"][1])),
           tuple(map(tuple, meta["pieces"][0])), tuple(map(tuple, meta["pieces"][1])))
    nc = _CACHE.get(key)
    if nc is None:
        nc = _build(meta)
        _CACHE[key] = nc
    res = run_bass_kernel_spmd(nc, in_maps, list(range(C)), trace=False)
    out = np.empty((N, F), np.float32)
    for ci in range(C):
        r = res.results[ci]
        xt = np.concatenate([r["outT0"], r["outT1"]], axis=0)  # [256, NL]
        out[ci * NL:(ci + 1) * NL] = xt.T
    return out


# revision 16
# speedup vs baseline: 1.0223x; 1.0141x over previous
"""ARMA GNN (3 layers, N=50000 nodes, E=800000 edges, F=256) on 8 TRN2 NeuronCores.

Strategy:
  - Shard nodes across 8 cores (6250 each); partition edges by destination owner
    so the segment-sum is local to each core.
  - All graph structure (edge lists, GCN norm) is known when the kernel is built,
    so the host precomputes: per-(src-half, dst-block) edge runs, int16 gather
    indices, and dense 128x128 "S matrices" (S[e, d] = norm_e one-hot on the dst
    column).  On device the whole message-passing step is:
        gather h[src] rows (SWDGE dma_gather)  ->  PSUM += S_chunk^T @ G_chunk
    i.e. gather + scale + segment-sum fused into TensorEngine matmuls.
  - The Q7 gather-descriptor generator costs ~3us per call regardless of size,
    so gather calls are packed to exactly 1024 indices, spanning dst-block
    boundaries (a straddling chunk simply feeds two matmuls with complementary
    zero columns).  Per-dst-block PSUM accumulators are spilled to SBUF between
    the two src-half passes and reloaded with an identity matmul.
  - Per layer: h = x @ Wi in bf16, AllGather'd in two chunks (src-half A then B,
    so half-A gathers overlap half-B's collective); message matmuls + x @ Wr
    accumulate in PSUM; transposed epilogue fuses ReLU+bias on the Act engine.
    x lives feature-major (xT) in SBUF between layers; host transposes output.
"""

import numpy as np
import ml_dtypes

import concourse.bass as bass
import concourse.bacc as bacc
import concourse.mybir as mybir
import concourse.tile as tile
from concourse.bass_utils import run_bass_kernel_spmd
from concourse.masks import make_identity

BF16 = ml_dtypes.bfloat16

# Problem constants (hardcoded per harness contract).
N = 50000
E = 800000
F = 256
L = 3
C = 8                     # cores
NL = N // C               # nodes per core = 6250
NB = (NL + 127) // 128    # dst blocks per core = 49
SA = 4096                 # local rows in src-half A (32 blocks)
SB_ = NL - SA             # local rows in src-half B = 2154 (17 blocks, last 106)
TBL = (C * SA, C * SB_)   # gather tables (32768, 17232) — int16-safe
NPAD = NB * 128           # padded local node count = 6272
GRING = 64                # G ring slots (chunks)


def _blkw(i):
    return NL - i * 128 if i == NB - 1 else 128


def _preprocess(x, edge_index, edge_attr, W_init, W_root, bias):
    """Host-side graph preprocessing. Returns (meta, per-core input maps)."""
    x = np.asarray(x, np.float32)
    ei = np.asarray(edge_index, np.int64)
    w = np.asarray(edge_attr, np.float32)
    W_init = np.asarray(W_init, np.float32)
    W_root = np.asarray(W_root, np.float32)
    bias = np.asarray(bias, np.float32)
    src, dst = ei[0], ei[1]

    deg = np.bincount(dst, weights=w.astype(np.float64), minlength=N).astype(np.float32)
    with np.errstate(divide="ignore"):
        dinv = np.where(deg > 0, 1.0 / np.sqrt(deg), 0.0).astype(np.float32)
    norm = (dinv[src] * w * dinv[dst]).astype(np.float32)

    core = dst // NL
    dloc = dst % NL
    db = dloc // 128
    dcol = dloc % 128
    sowner = src // NL
    sloc = src % NL
    half = (sloc >= SA).astype(np.int64)
    tbl = np.where(half == 0, sowner * SA + sloc, sowner * SB_ + (sloc - SA))

    # sort edges by (core, half, db, tbl)
    order = np.lexsort((tbl, db, half, core))
    g_core, g_half, g_db = core[order], half[order], db[order]
    g_tbl, g_norm, g_dcol = tbl[order], norm[order], dcol[order]

    # per-(core, half, db) counts -> unified run lengths (max over cores, SPMD)
    cnt = np.zeros((C, 2, NB), np.int64)
    np.add.at(cnt, (g_core, g_half, g_db), 1)
    Lhb = cnt.max(axis=0)                      # [2, NB]

    run_key = (g_core * 2 + g_half) * NB + g_db
    starts = np.searchsorted(run_key, np.arange(C * 2 * NB))
    ends = np.append(starts[1:], len(run_key))

    # unified layout: per half, concatenated padded (h, db) runs; each half's
    # total padded up to a chunk (128) multiple
    off_hb = np.zeros((2, NB), np.int64)
    POS = [0, 0]
    for h in range(2):
        p = 0
        for b in range(NB):
            off_hb[h, b] = p
            p += int(Lhb[h, b])
        POS[h] = -(-p // 128) * 128
    NCHUNK = [POS[0] // 128, POS[1] // 128]

    # pieces: (chunk, db, s0, s1) — intersection of a 128-chunk with a run.
    # Emitted in position order (runs are disjoint ascending intervals).
    pieces = [[], []]
    for h in range(2):
        for b in range(NB):
            lo, hi = int(off_hb[h, b]), int(off_hb[h, b] + Lhb[h, b])
            if lo == hi:
                continue
            for cc in range(lo // 128, (hi - 1) // 128 + 1):
                s0 = max(lo, cc * 128) - cc * 128
                s1 = min(hi, (cc + 1) * 128) - cc * 128
                pieces[h].append((cc, b, s0, s1))
        # position-ordered check (required for contiguous S streaming)
        ppos = [cc * 128 + s0 for (cc, b, s0, s1) in pieces[h]]
        assert all(ppos[i] < ppos[i + 1] for i in range(len(ppos) - 1))
    NPIECE = [len(pieces[0]), len(pieces[1])]
    SW = (NPIECE[0] + NPIECE[1]) * 128
    WT = (POS[0] + POS[1]) // 16

    # gather calls per half: groups of <=8 chunks
    calls = [[], []]
    for h in range(2):
        cc = 0
        while cc < NCHUNK[h]:
            n = min(8, NCHUNK[h] - cc)
            calls[h].append((cc, n))
            cc += n
    # max pieces per call (St tile sizing)
    maxpc = 1
    for h in range(2):
        cp = {}
        for pi, (cc, b, s0, s1) in enumerate(pieces[h]):
            cp.setdefault(cc // 8, []).append(pi)
        maxpc = max([maxpc] + [len(v) for v in cp.values()])

    in_maps = []
    for ci in range(C):
        idx_all = np.zeros((128, WT), np.int16)
        s_all = np.zeros((128, SW), BF16)
        for h in range(2):
            base = POS[0] // 16 if h == 1 else 0
            for b in range(NB):
                k = (ci * 2 + h) * NB + b
                a, e = starts[k], ends[k]
                ne = e - a
                if ne == 0:
                    continue
                j = int(off_hb[h, b]) + np.arange(ne)
                t16 = g_tbl[a:e].astype(np.int16)
                cols = base + j // 16
                rows = j % 16
                for grp in range(8):
                    idx_all[grp * 16 + rows, cols] = t16
        poff = 0
        for h in range(2):
            for (cc, b, s0, s1) in pieces[h]:
                k = (ci * 2 + h) * NB + b
                a, e = starts[k], ends[k]
                ne = e - a
                glo = cc * 128 + s0 - int(off_hb[h, b])
                ghi = cc * 128 + s1 - int(off_hb[h, b])
                lo, hi = max(0, glo), min(ne, ghi)
                if hi > lo:
                    jj = np.arange(lo, hi)
                    prow = (int(off_hb[h, b]) + jj) % 128
                    s_all[prow, poff + g_dcol[a + lo:a + hi]] = \
                        g_norm[a + lo:a + hi].astype(BF16)
                poff += 128
        xT = np.zeros((256, NPAD), BF16)
        xT[:, :NL] = x[ci * NL:(ci + 1) * NL].T.astype(BF16)
        in_maps.append(dict(
            xT=xT,
            idx_all=idx_all,
            s_all=s_all,
            wi=W_init.reshape(L, 2, 128, F).astype(BF16),
            wr=W_root.reshape(L, 2, 128, F).astype(BF16),
            bias_c=np.ascontiguousarray(
                bias.reshape(L * 2, 128).T.astype(np.float32)),  # [128, L*2]
        ))

    meta = dict(pieces=pieces, calls=calls, NCHUNK=NCHUNK, NPIECE=NPIECE,
                POS=POS, WT=WT, SW=SW, maxpc=maxpc)
    return meta, in_maps


def _h_block(nc, psp, wp, l, nb, xsrc, wi_sb, hb):
    """Emit h = x @ Wi for one node block of layer l into its bounce buffer."""
    bf = mybir.dt.bfloat16
    f32 = mybir.dt.float32
    hbA, hbB = hb
    w = _blkw(nb)
    c0 = nb * 128
    ph = psp.tile([128, F], f32, tag="ph", bufs=3, name=f"ph{l}_{nb}")
    for g in range(2):
        nc.tensor.matmul(out=ph[:w, :], lhsT=xsrc[g][:, c0:c0 + w],
                         rhs=wi_sb[l][g][:], start=(g == 0), stop=(g == 1))
    hsb = wp.tile([128, F], bf, tag="hsb", bufs=6, name=f"hsb{l}_{nb}")
    nc.vector.tensor_copy(hsb[:w, :], ph[:w, :])
    if nb < SA // 128:
        nc.sync.dma_start(out=hbA[c0:c0 + w, :], in_=hsb[:w, :])
    else:
        r0 = c0 - SA
        nc.sync.dma_start(out=hbB[r0:r0 + w, :], in_=hsb[:w, :])


def _epilogue(nc, psp, wp, t, b, l, ident, bias_sb, xw_, outT, nxt):
    bf = mybir.dt.bfloat16
    f32 = mybir.dt.float32
    wd = _blkw(b)
    c0 = b * 128
    cp = wp.tile([128, F], bf, tag="cp", bufs=3, name=f"cp{l}_{b}")
    nc.vector.tensor_copy(cp[:], t[:])
    for g in range(2):
        pt = psp.tile([128, 128], bf, tag="pt", bufs=2, name=f"pt{l}_{b}_{g}")
        nc.tensor.transpose(out=pt[:], in_=cp[:, g * 128:(g + 1) * 128],
                            identity=ident[:])
        bcol = bias_sb[l * 2 + g][:]
        if l < L - 1:
            nc.scalar.activation(
                out=xw_[g][:, c0:c0 + wd], in_=pt[:, :wd],
                func=mybir.ActivationFunctionType.Relu, bias=bcol)
        else:
            ot = wp.tile([128, 128], f32, tag="ot", bufs=2, name=f"ot{l}_{b}_{g}")
            nc.scalar.activation(
                out=ot[:, :wd], in_=pt[:, :wd],
                func=mybir.ActivationFunctionType.Relu, bias=bcol)
            nc.sync.dma_start(out=outT[g][:, c0:c0 + wd], in_=ot[:, :wd])
    if l < L - 1:
        # queue next layer's h for this block; emitted with a lag so the PE
        # doesn't stall waiting on this epilogue's Act write
        nxt["pend"].append(b)
        _flush_h(nc, psp, wp, l, xw_, nxt, lag=4)


def _flush_h(nc, psp, wp, l, xw_, nxt, lag):
    while len(nxt["pend"]) > lag:
        b = nxt["pend"].pop(0)
        _h_block(nc, psp, wp, l + 1, b, xw_, nxt["wi_sb"], nxt["hb"])
        if b < SA // 128:
            nxt["remA"] -= 1
            if nxt["remA"] == 0:
                nc.gpsimd.collective_compute(
                    "AllGather", mybir.AluOpType.bypass,
                    replica_groups=nxt["groups"], ins=[nxt["hb"][0][:]],
                    outs=[nxt["hg"][0][:]])
        else:
            nxt["remB"] -= 1
            if nxt["remB"] == 0:
                nc.gpsimd.collective_compute(
                    "AllGather", mybir.AluOpType.bypass,
                    replica_groups=nxt["groups"], ins=[nxt["hb"][1][:]],
                    outs=[nxt["hg"][1][:]])


def _build(meta):
    pieces, calls = meta["pieces"], meta["calls"]
    POS, WT, SW = meta["POS"], meta["WT"], meta["SW"]
    NPIECE, maxpc = meta["NPIECE"], meta["maxpc"]
    bf = mybir.dt.bfloat16
    f32 = mybir.dt.float32

    nc = bacc.Bacc("TRN2", target_bir_lowering=False, debug=False, num_devices=C,
                   num_swdge_queues=4, dynamic_dma_scratch_size=32768)
    xT_p = nc.dram_tensor("xT", [256, NPAD], bf, kind="ExternalInput")
    idx_p = nc.dram_tensor("idx_all", [128, WT], mybir.dt.int16, kind="ExternalInput")
    s_p = nc.dram_tensor("s_all", [128, SW], bf, kind="ExternalInput")
    wi_p = nc.dram_tensor("wi", [L, 2, 128, F], bf, kind="ExternalInput")
    wr_p = nc.dram_tensor("wr", [L, 2, 128, F], bf, kind="ExternalInput")
    bias_p = nc.dram_tensor("bias_c", [128, L * 2], f32, kind="ExternalInput")
    outT = [nc.dram_tensor(f"outT{g}", [128, NL], f32, kind="ExternalOutput")
            for g in range(2)]

    groups = [list(range(C))]
    # chunk -> [(local_pi, cc, b, s0, s1)] and db -> last local_pi, per half
    chunk_pieces = [{}, {}]
    db_last = [[None] * NB, [None] * NB]
    db_any = [[False] * NB, [False] * NB]
    for h in range(2):
        for pi, (cc, b, s0, s1) in enumerate(pieces[h]):
            chunk_pieces[h].setdefault(cc, []).append((pi, cc, b))
            db_last[h][b] = pi
            db_any[h][b] = True

    with tile.TileContext(nc) as tc:
        with (
            tc.tile_pool(name="persist", bufs=1) as pp,
            tc.tile_pool(name="dram", bufs=2, space="DRAM") as dp,
            tc.tile_pool(name="psum", bufs=3, space="PSUM") as psp,
            tc.tile_pool(name="work", bufs=3) as wp,
        ):
            ident = pp.tile([128, 128], bf)
            make_identity(nc, ident[:])
            idx_sb = pp.tile([128, WT], mybir.dt.int16)
            nc.sync.dma_start(out=idx_sb[:], in_=idx_p[:])
            bias_sb = [pp.tile([128, 1], f32, name=f"bias{c}") for c in range(L * 2)]
            for c_ in range(L * 2):
                nc.sync.dma_start(out=bias_sb[c_][:], in_=bias_p[:, c_:c_ + 1])
            wi_sb = [[pp.tile([128, F], bf, name=f"wi{l}{g}") for g in range(2)]
                     for l in range(L)]
            wr_sb = [[pp.tile([128, F], bf, name=f"wr{l}{g}") for g in range(2)]
                     for l in range(L)]
            for l in range(L):
                for g in range(2):
                    nc.sync.dma_start(out=wi_sb[l][g][:], in_=wi_p[l, g])
                    nc.sync.dma_start(out=wr_sb[l][g][:], in_=wr_p[l, g])
            xa = [pp.tile([128, NPAD], bf, name=f"xa{g}") for g in range(2)]
            xb = [pp.tile([128, NPAD], bf, name=f"xb{g}") for g in range(2)]
            for g in range(2):
                nc.sync.dma_start(out=xa[g][:], in_=xT_p[g * 128:(g + 1) * 128, :])
                if NPAD > NL:
                    nc.gpsimd.memset(xb[g][:, NL:], 0.0)
            acc_sb = pp.tile([128, NB * F], bf)   # pass A -> pass B spill

            # per-layer bounce/gather-table tiles (bufs=2 ping-pong)
            hb = []
            hg = []
            for l in range(L):
                hb.append((dp.tile([SA, F], bf, tag="hbA", name=f"hbA{l}"),
                           dp.tile([SB_, F], bf, tag="hbB", name=f"hbB{l}")))
                hg.append((dp.tile([TBL[0], F], bf, addr_space="Shared",
                                   tag="hgA", name=f"hgA{l}"),
                           dp.tile([TBL[1], F], bf, addr_space="Shared",
                                   tag="hgB", name=f"hgB{l}")))

            # prologue: layer 0 h-phase + collectives
            for nb in range(NB):
                _h_block(nc, psp, wp, 0, nb, xa, wi_sb, hb[0])
                if nb == SA // 128 - 1:
                    nc.gpsimd.collective_compute(
                        "AllGather", mybir.AluOpType.bypass,
                        replica_groups=groups, ins=[hb[0][0][:]],
                        outs=[hg[0][0][:]])
            nc.gpsimd.collective_compute(
                "AllGather", mybir.AluOpType.bypass,
                replica_groups=groups, ins=[hb[0][1][:]], outs=[hg[0][1][:]])

            qn = [0]
            for l in range(L):
                xr_ = xa if l % 2 == 0 else xb
                xw_ = xb if l % 2 == 0 else xa
                hgA, hgB = hg[l]
                if l < L - 1:
                    nxt = dict(wi_sb=wi_sb, hb=hb[l + 1], hg=hg[l + 1],
                               remA=SA // 128, remB=NB - SA // 128,
                               groups=groups, pend=[])
                else:
                    nxt = None

                # ---- message passing: pass A then pass B ----
                Gr = pp.tile([128, GRING, F], bf, name=f"Gr{l}", tag="Gr")
                pa = {}
                for h in range(2):
                    hgx = hgA if h == 0 else hgB
                    ibase = POS[0] // 16 if h == 1 else 0
                    pbase = NPIECE[0] if h == 1 else 0
                    for (clo, ncnk) in calls[h]:
                        slot0 = clo % GRING
                        nidx = ncnk * 128
                        nc.gpsimd.dma_gather(
                            out_ap=Gr[:, slot0:slot0 + ncnk, :], in_ap=hgx[:],
                            idxs_ap=idx_sb[:, ibase + clo * 8:
                                           ibase + (clo + ncnk) * 8],
                            num_idxs=nidx, num_idxs_reg=nidx,
                            elem_size=F, queue_num=qn[0] % 4)
                        qn[0] += 1
                        plist = []
                        for cc in range(clo, clo + ncnk):
                            plist += chunk_pieces[h].get(cc, [])
                        if not plist:
                            continue
                        p0 = plist[0][0]
                        np_ = len(plist)
                        St = wp.tile([128, maxpc * 128], bf, tag="St", bufs=4,
                                     name=f"St{l}_{h}_{clo}")
                        nc.sync.dma_start(
                            out=St[:, :np_ * 128],
                            in_=s_p[:, (pbase + p0) * 128:(pbase + p0 + np_) * 128])
                        for k, (pi, cc, b) in enumerate(plist):
                            if b not in pa:
                                t = psp.tile([128, F], f32, tag="pa", bufs=2,
                                             name=f"pa{l}_{h}_{b}")
                                pa[b] = t
                                if h == 1 and db_any[0][b]:
                                    nc.tensor.matmul(
                                        out=t[:], lhsT=ident[:],
                                        rhs=acc_sb[:, b * F:(b + 1) * F],
                                        start=True, stop=False)
                                else:
                                    for g in range(2):
                                        nc.tensor.matmul(
                                            out=t[:],
                                            lhsT=xr_[g][:, b * 128:b * 128 + 128],
                                            rhs=wr_sb[l][g][:],
                                            start=(g == 0), stop=False)
                            last_piece = (pi == db_last[h][b])
                            nc.tensor.matmul(
                                out=pa[b][:],
                                lhsT=St[:, k * 128:(k + 1) * 128],
                                rhs=Gr[:, slot0 + (cc - clo), :],
                                start=False, stop=last_piece)
                            if last_piece:
                                t = pa.pop(b)
                                if h == 0 and db_any[1][b]:
                                    nc.vector.tensor_copy(
                                        acc_sb[:, b * F:(b + 1) * F], t[:])
                                else:
                                    _epilogue(nc, psp, wp, t, b, l, ident,
                                              bias_sb, xw_, outT, nxt)
                assert not pa
                if nxt is not None:
                    _flush_h(nc, psp, wp, l, xw_, nxt, lag=0)
                # dbs with no edges at all (xr + bias + relu only)
                for b in range(NB):
                    if not db_any[0][b] and not db_any[1][b]:
                        t = psp.tile([128, F], f32, tag="pa", bufs=2,
                                     name=f"paz{l}_{b}")
                        for g in range(2):
                            nc.tensor.matmul(
                                out=t[:], lhsT=xr_[g][:, b * 128:b * 128 + 128],
                                rhs=wr_sb[l][g][:], start=(g == 0), stop=(g == 1))
                        _epilogue(nc, psp, wp, t, b, l, ident, bias_sb,
                                  xw_, outT, nxt)
    nc.compile()
    return nc


_CACHE = {}


def kernel(**inputs):
    meta, in_maps = _preprocess(**inputs)
    key = (tuple(map(tuple, meta["calls"][0])), tuple(map(tuple, meta["calls"][1])),
           tuple(map(tuple, meta["pieces"][0])), tuple(map(tuple, meta["pieces"][1])))
    nc = _CACHE.get(key)
    if nc is None:
        nc = _build(meta)
        _CACHE[key] = nc
    res = run_bass_kernel_spmd(nc, in_maps, list(range(C)), trace=False)
    out = np.empty((N, F), np.float32)
    for ci in range(C):
        r = res.results[ci]
        xt = np.concatenate([r["outT0"], r["outT1"]], axis=0)  # [256, NL]
        out[ci * NL:(ci + 1) * NL] = xt.T
    return out
